# revision 1
# baseline (speedup 1.0000x reference)
"""Bass/Tile GATv2 layer kernel for TRN2, dst-sharded across cores.

Structure per core (one SPMD program, per-core data in in_maps):
  Phase 1: xlc{0,1} = x_compact @ wl      (compact source tables per subphase;
           x_compact is a host-side row-gather of x over the unique sources
           referenced by that subphase's edges, so int16 dma_gather indices fit)
  Phase 2: xr_loc = x_loc @ wr            (local dst nodes, kept in SBUF bf16)
           res = x_loc @ lin_w + lin_b    (layer 1 only)
  Phase 3: per dst-tile (128 dst slots):
           - one dma_gather of xl_compact rows for NB*128 edges
           - per 128-edge block: one-hot expand xr, add xl, leaky-relu,
             att-weighted per-head score, exp, segment-sum via one-hot matmul
             into PSUM accumulators (U, D)
           - epilogue: out = mean_h(U/D) + bias [, relu] [, + residual]
"""
from contextlib import ExitStack
from dataclasses import dataclass

import numpy as np

import concourse.bass as bass
import concourse.tile as tile
from concourse import bacc, mybir
from concourse.masks import make_identity

F32 = mybir.dt.float32
BF16 = mybir.dt.bfloat16
I16 = mybir.dt.int16
AF = mybir.ActivationFunctionType
ALU = mybir.AluOpType
AX = mybir.AxisListType
P = 128


@dataclass
class LayerCfg:
    Fin: int            # contraction dim of input features (128 L1, 64 L2)
    H: int              # heads
    CH: int             # per-head channels (64 L1, 32 L2)
    NT: int             # dst tiles per core
    NB: int             # edge blocks per tile (uniform across cores/tiles)
    NT0: int            # tiles in subphase 0 (rest in subphase 1)
    ROWPAD0: int        # compact-table rows, subphase 0 (128-mult, uniform)
    ROWPAD1: int        # compact-table rows, subphase 1 (0 if single subphase)
    relu: bool = False
    calc_residual: bool = False
    add_residual: bool = False
    RESC: int = 32
    neg_slope: float = 0.2
    chunk: int = 2048   # xTc column chunk for phase 1
    bf16: bool = False  # bf16 tables + matmul operands (fast path)
    exb_pool: bool = False  # materialize exp broadcast on GpSimd (helps L2)
    bufs_big: bool = False  # larger working pools (helps L2)
    repeat: int = 1     # emit the whole compute body N times (timing aid)

    @property
    def HC(self):
        return self.H * self.CH


def build_layer(nc: bacc.Bacc, cfg: LayerCfg):
    """Declare tensors and emit the full layer program into nc."""
    HC, H, CH, Fin, NT, NB = cfg.HC, cfg.H, cfg.CH, cfg.Fin, cfg.NT, cfg.NB
    NLOC = NT * P
    NIDX = NB * P            # gathered rows per tile
    IDXW = NIDX // 16        # idx16 columns per tile
    TDT = BF16 if cfg.bf16 else F32   # table / matmul-operand dtype

    dt = nc.dram_tensor
    xTc0_t = dt("xTc0", (Fin, cfg.ROWPAD0), TDT, kind="ExternalInput")
    if cfg.ROWPAD1:
        xTc1_t = dt("xTc1", (Fin, cfg.ROWPAD1), TDT, kind="ExternalInput")
    xTloc_t = dt("xTloc", (Fin, NLOC), F32, kind="ExternalInput")
    wl_t = dt("wl", (Fin, HC), TDT, kind="ExternalInput")
    wr_t = dt("wr", (Fin, HC), F32, kind="ExternalInput")
    attb_t = dt("attb", (P, HC), TDT, kind="ExternalInput")
    bbc_t = dt("bbc", (P, CH), F32, kind="ExternalInput")
    idx16_t = dt("idx16", (P, NT * IDXW), I16, kind="ExternalInput")
    erelc_t = dt("erelc", (P, NT * NB), F32, kind="ExternalInput")
    iotar_t = dt("iotar", (P, P), F32, kind="ExternalInput")
    if cfg.calc_residual:
        linw_t = dt("linw", (Fin, cfg.RESC), F32, kind="ExternalInput")
        linb_t = dt("linb", (P, cfg.RESC), F32, kind="ExternalInput")
        resout_t = dt("resout", (NLOC, cfg.RESC), F32, kind="ExternalOutput")
    if cfg.add_residual:
        resin_t = dt("resin", (NLOC, cfg.RESC), F32, kind="ExternalInput")
    out_t = dt("out", (NLOC, CH), F32, kind="ExternalOutput")
    xlc0_t = dt("xlc0", (cfg.ROWPAD0, HC), TDT)   # internal scratch
    xlc1_t = dt("xlc1", (cfg.ROWPAD1, HC), TDT) if cfg.ROWPAD1 else None

    with tile.TileContext(nc) as tc, ExitStack() as ctx:
        cpool = ctx.enter_context(tc.tile_pool(name="const", bufs=1))
        xt_pool = ctx.enter_context(tc.tile_pool(name="xt", bufs=3))
        cp_pool = ctx.enter_context(tc.tile_pool(name="cp", bufs=2))
        g_pool = ctx.enter_context(tc.tile_pool(name="g", bufs=2))
        _b = (4, 4, 6) if cfg.bufs_big else (3, 3, 4)
        oh_pool = ctx.enter_context(tc.tile_pool(name="oh", bufs=_b[0]))
        lr_pool = ctx.enter_context(tc.tile_pool(name="lr", bufs=_b[1]))
        sm_pool = ctx.enter_context(tc.tile_pool(name="sm", bufs=_b[2]))
        _pb = (2, 2) if cfg.bufs_big else (3, 1)
        ps_pool = ctx.enter_context(tc.tile_pool(name="ps", bufs=_pb[0], space="PSUM"))
        psb_pool = ctx.enter_context(tc.tile_pool(name="psb", bufs=_pb[1], space="PSUM"))
        psu_pool = ctx.enter_context(tc.tile_pool(name="psu", bufs=2, space="PSUM"))
        psd_pool = ctx.enter_context(tc.tile_pool(name="psd", bufs=2, space="PSUM"))

        # ---- constants ----
        wl_sb = cpool.tile([Fin, HC], TDT)
        nc.sync.dma_start(out=wl_sb[:], in_=wl_t[:, :])
        wr_sb = cpool.tile([Fin, HC], F32)
        nc.sync.dma_start(out=wr_sb[:], in_=wr_t[:, :])
        attb_sb = cpool.tile([P, HC], TDT)
        nc.sync.dma_start(out=attb_sb[:], in_=attb_t[:, :])
        bbc_sb = cpool.tile([P, CH], F32)
        nc.sync.dma_start(out=bbc_sb[:], in_=bbc_t[:, :])
        iotar_sb = cpool.tile([P, P], F32)
        nc.sync.dma_start(out=iotar_sb[:], in_=iotar_t[:, :])
        ident_sb = cpool.tile([P, P], TDT)
        make_identity(nc, ident_sb[:])
        idx16_sb = cpool.tile([P, NT * IDXW], I16)
        nc.sync.dma_start(out=idx16_sb[:], in_=idx16_t[:, :])
        erelc_sb = cpool.tile([P, NT * NB], F32)
        nc.sync.dma_start(out=erelc_sb[:], in_=erelc_t[:, :])
        xtloc_sb = cpool.tile([Fin, NLOC], F32)
        nc.sync.dma_start(out=xtloc_sb[:], in_=xTloc_t[:, :])
        if cfg.calc_residual:
            linw_sb = cpool.tile([Fin, cfg.RESC], F32)
            nc.sync.dma_start(out=linw_sb[:], in_=linw_t[:, :])
            linb_sb = cpool.tile([P, cfg.RESC], F32)
            nc.sync.dma_start(out=linb_sb[:], in_=linb_t[:, :])
            res_acc = cpool.tile([P, NT * cfg.RESC], F32)
        if cfg.add_residual:
            res_sb = cpool.tile([P, NT * cfg.RESC], F32)
            nc.sync.dma_start(
                out=res_sb[:].rearrange("p (t c) -> p t c", t=NT),
                in_=resin_t[:, :].rearrange("(t p) c -> p t c", p=P),
            )
        h_acc = cpool.tile([P, NT * CH], F32)

        # ---- phase 1: xlc = x_compact @ wl (per subphase) ----
        flip = 0
        for _rep in range(cfg.repeat):
         for xTc_t, xlc_t, rows in ((xTc0_t, xlc0_t, cfg.ROWPAD0),
                                    (xTc1_t, xlc1_t, cfg.ROWPAD1) if cfg.ROWPAD1
                                    else (None, None, 0)):
             c0 = 0
             while c0 < rows:
                 csz = min(cfg.chunk, rows - c0)
                 xt_sb = xt_pool.tile([Fin, csz], TDT, tag="xt")
                 nc.sync.dma_start(out=xt_sb[:], in_=xTc_t[:, c0:c0 + csz])
                 nj = csz // P
                 ob = cp_pool.tile([P, nj * HC], TDT, tag="cp")
                 for j in range(nj):
                     ps = ps_pool.tile([P, HC], F32, tag="mm")
                     nc.tensor.matmul(ps[:], lhsT=xt_sb[:, j * P:(j + 1) * P],
                                      rhs=wl_sb[:], start=True, stop=True)
                     if flip % 2 == 0:
                         nc.vector.tensor_copy(ob[:, j * HC:(j + 1) * HC], ps[:])
                     else:
                         nc.scalar.copy(ob[:, j * HC:(j + 1) * HC], ps[:])
                     flip += 1
                 nc.sync.dma_start(
                     out=xlc_t[c0:c0 + csz, :].rearrange("(j p) c -> p j c", p=P),
                     in_=ob[:].rearrange("p (j c) -> p j c", j=nj))
                 c0 += csz

         # ---- phase 2: residual ----
         for t in range(NT):
             if cfg.calc_residual:
                 ps2 = psd_pool.tile([P, cfg.RESC], F32, tag="D")
                 nc.tensor.matmul(ps2[:], lhsT=xtloc_sb[:, t * P:(t + 1) * P],
                                  rhs=linw_sb[:], start=True, stop=True)
                 nc.vector.tensor_tensor(res_acc[:, t * cfg.RESC:(t + 1) * cfg.RESC],
                                         ps2[:], linb_sb[:], op=ALU.add)

         # ---- phase 3: edge processing ----
         for t in range(NT):
             src_tab = xlc0_t if t < cfg.NT0 else xlc1_t
             # xr rows for this dst tile, f32, computed on the fly
             psx = ps_pool.tile([P, HC], F32, tag="mm")
             nc.tensor.matmul(psx[:], lhsT=xtloc_sb[:, t * P:(t + 1) * P],
                              rhs=wr_sb[:], start=True, stop=True)
             xrt = lr_pool.tile([P, HC], TDT, tag="xrt")
             nc.vector.tensor_copy(xrt[:], psx[:])
             xlg = g_pool.tile([P, NB * HC], TDT, tag="g")
             nc.gpsimd.dma_gather(
                 out_ap=xlg[:].rearrange("p (b c) -> p b c", b=NB),
                 in_ap=src_tab[:, :],
                 idxs_ap=idx16_sb[:, t * IDXW:(t + 1) * IDXW],
                 num_idxs=NIDX,
                 num_idxs_reg=NIDX,
                 elem_size=HC,
                 single_packet=False,
             )
             U = psu_pool.tile([P, HC], F32, tag="U")
             D = psd_pool.tile([P, H], F32, tag="D")
             for b in range(NB):
                 gb = t * NB + b
                 oh = oh_pool.tile([P, P], TDT, tag="oh")
                 nc.vector.tensor_tensor(oh[:], erelc_sb[:, gb:gb + 1].to_broadcast([P, P]),
                                         iotar_sb[:], op=ALU.is_equal)
                 bc = psb_pool.tile([P, P], TDT, tag="bc")
                 nc.tensor.transpose(bc[:], oh[:], ident_sb[:])
                 ohT = oh_pool.tile([P, P], TDT, tag="ohT")
                 nc.scalar.copy(ohT[:], bc[:])
                 s = ps_pool.tile([P, HC], F32, tag="mm")
                 nc.tensor.matmul(s[:], lhsT=ohT[:], rhs=xrt[:],
                                  start=True, stop=False)
                 nc.tensor.matmul(s[:], lhsT=ident_sb[:], rhs=xlg[:, b * HC:(b + 1) * HC],
                                  start=False, stop=True)
                 lr = lr_pool.tile([P, HC], TDT, tag="lr")
                 nc.scalar.activation(lr[:], s[:], AF.Prelu, alpha=cfg.neg_slope)
                 nc.vector.tensor_tensor(lr[:], lr[:], attb_sb[:], op=ALU.mult)
                 e = sm_pool.tile([P, H], F32, tag="e")
                 nc.vector.tensor_reduce(
                     e[:], lr[:].rearrange("p (h c) -> p h c", h=H),
                     axis=AX.X, op=ALU.add)
                 ex = sm_pool.tile([P, H], TDT, tag="ex")
                 nc.scalar.activation(ex[:], e[:], AF.Exp)
                 if cfg.exb_pool:
                     exb = oh_pool.tile([P, HC], TDT, tag="exb")
                     nc.gpsimd.tensor_copy(exb[:], ex[:].to_broadcast([P, H, CH]))
                     ex_in = exb[:]
                 else:
                     ex_in = ex[:].to_broadcast([P, H, CH])
                 nc.vector.tensor_tensor(
                     xlg[:, b * HC:(b + 1) * HC],
                     xlg[:, b * HC:(b + 1) * HC],
                     ex_in,
                     op=ALU.mult)
                 nc.tensor.matmul(U[:], lhsT=oh[:], rhs=xlg[:, b * HC:(b + 1) * HC],
                                  start=(b == 0), stop=(b == NB - 1))
                 nc.tensor.matmul(D[:], lhsT=oh[:], rhs=ex[:],
                                  start=(b == 0), stop=(b == NB - 1))
             # epilogue for tile t
             dsafe = sm_pool.tile([P, H], F32, tag="dsafe")
             nc.vector.tensor_scalar_max(dsafe[:], D[:], 1e-30)
             rcp = sm_pool.tile([P, H], F32, tag="rcp")
             nc.vector.reciprocal(rcp[:], dsafe[:])
             au = lr_pool.tile([P, HC], F32, tag="au")
             nc.vector.tensor_tensor(au[:], U[:],
                                     rcp[:].to_broadcast([P, H, CH]),
                                     op=ALU.mult)
             hm = sm_pool.tile([P, CH], F32, tag="hm")
             nc.vector.tensor_reduce(
                 hm[:], au[:].rearrange("p (h c) -> p c h", h=H),
                 axis=AX.X, op=ALU.add)
             t1 = sm_pool.tile([P, CH], F32, tag="t1")
             nc.vector.tensor_scalar_mul(t1[:], hm[:], 1.0 / H)
             nc.vector.tensor_tensor(t1[:], t1[:], bbc_sb[:], op=ALU.add)
             if cfg.add_residual:
                 nc.vector.tensor_tensor(t1[:], t1[:],
                                         res_sb[:, t * cfg.RESC:(t + 1) * cfg.RESC],
                                         op=ALU.add)
             if cfg.relu:
                 nc.scalar.activation(h_acc[:, t * CH:(t + 1) * CH], t1[:], AF.Relu)
             else:
                 nc.vector.tensor_copy(h_acc[:, t * CH:(t + 1) * CH], t1[:])

        # ---- final stores ----
        nc.sync.dma_start(
            out=out_t[:, :].rearrange("(t p) c -> p t c", p=P),
            in_=h_acc[:].rearrange("p (t c) -> p t c", t=NT),
        )
        if cfg.calc_residual:
            nc.sync.dma_start(
                out=resout_t[:, :].rearrange("(t p) c -> p t c", p=P),
                in_=res_acc[:].rearrange("p (t c) -> p t c", t=NT),
            )
    return nc


# ---------------------------------------------------------------------------
# Host-side preprocessing
# ---------------------------------------------------------------------------

def preprocess_edges(edge_index: np.ndarray, n: int, ncores: int, dtile: int = P,
                     nsub: int = 2, int16_cap: int = 32000):
    """Shard edges by dst; per core: dst tiles of 128, uniform NB blocks/tile,
    compact int16 source indexing per subphase.

    Returns (metas, layout) where metas[c] has:
      idx16  [128, NT*NB*8]  int16 (wrapped-by-16, replicated to 128)
      erelc  [128, NT*NB]    f32 rel-dst per edge (-1 pad)
      usrc   [nsub] list of unique source-node arrays (host gathers x rows)
    """
    loops = np.arange(n, dtype=np.int64)
    src = np.concatenate([edge_index[0].astype(np.int64), loops])
    dst = np.concatenate([edge_index[1].astype(np.int64), loops])
    nloc = -(-n // ncores)
    nloc_pad = -(-nloc // dtile) * dtile
    NT = nloc_pad // dtile
    NT0 = (NT + 1) // 2 if nsub == 2 else NT
    order = np.argsort(dst, kind='stable')
    src, dst = src[order], dst[order]

    per_core = []
    NB = 1
    for c in range(ncores):
        d0 = c * nloc
        d1 = min((c + 1) * nloc, n)
        sel = (dst >= d0) & (dst < d1)
        s_c, d_c = src[sel], dst[sel] - d0
        tiles = []
        for t in range(NT):
            tsel = (d_c >= t * dtile) & (d_c < (t + 1) * dtile)
            tiles.append((s_c[tsel], d_c[tsel] - t * dtile))
            NB = max(NB, -(-len(tiles[-1][0]) // P))
        per_core.append(tiles)

    NIDX = NB * P
    IDXW = NIDX // 16
    metas = []
    rowmax = [0] * nsub
    for c in range(ncores):
        erelc = np.full((P, NT * NB), -1.0, np.float32)
        idxflat = np.zeros((NT, NIDX), np.int64)
        usrcs = []
        for s in range(nsub):
            tlo, thi = (0, NT0) if s == 0 else (NT0, NT)
            allsrc = np.concatenate([per_core[c][t][0] for t in range(tlo, thi)]) \
                if thi > tlo else np.zeros(0, np.int64)
            usrc = np.unique(allsrc)
            assert len(usrc) < int16_cap, f"int16 cap exceeded: {len(usrc)}"
            usrcs.append(usrc)
            lookup = {v: i for i, v in enumerate(usrc)}
            for t in range(tlo, thi):
                s_t, rel_t = per_core[c][t]
                ne = len(s_t)
                comp = np.fromiter((lookup[v] for v in s_t), np.int64, ne)
                idxflat[t, :ne] = comp          # pads stay 0 (valid row)
                for b in range(NB):
                    gb = t * NB + b
                    lo, hi = b * P, min((b + 1) * P, ne)
                    if hi > lo:
                        erelc[:hi - lo, gb] = rel_t[lo:hi]
        # wrap idx: i -> partition i%16, col i//16; replicate to 128
        w = idxflat.reshape(NT, IDXW, 16).transpose(0, 2, 1)  # [NT, 16, IDXW]
        idx16 = np.tile(w, (1, 8, 1)).transpose(1, 0, 2).reshape(P, NT * IDXW)
        metas.append(dict(idx16=idx16.astype(np.int16), erelc=erelc, usrc=usrcs))
        for s in range(nsub):
            rowmax[s] = max(rowmax[s], len(metas[c]['usrc'][s]))
    rowpad = [-(-r // P) * P for r in rowmax]
    layout = dict(NT=NT, NB=NB, NT0=NT0, nloc=nloc, nloc_pad=nloc_pad,
                  ROWPAD0=rowpad[0], ROWPAD1=rowpad[1] if nsub == 2 else 0)
    return metas, layout


def host_constants(HC: int, CH: int, att: np.ndarray, bias: np.ndarray):
    """attb [128, HC], bbc [128, CH], iotar constant."""
    attb = np.tile(att.reshape(1, HC).astype(np.float32), (P, 1))
    bbc = np.tile(bias.reshape(1, CH).astype(np.float32), (P, 1))
    iotar = np.tile(np.arange(P, dtype=np.float32).reshape(1, P), (P, 1))
    return attb, bbc, iotar


# ---------------------------------------------------------------------------
# Top-level kernel entry: full inputs -> full output, 8 NeuronCores
# ---------------------------------------------------------------------------
import ml_dtypes

_BF16NP = ml_dtypes.bfloat16
N_NODES = 50000
F_IN = 128
N_HEADS = 8
C_HID = 64
K_OUT = 32
NCORES = 8

_compiled_cache = {}


def _build_programs(lay):
    key = (lay['NT'], lay['NB'], lay['NT0'], lay['ROWPAD0'], lay['ROWPAD1'])
    if key in _compiled_cache:
        return _compiled_cache[key]
    cfg1 = LayerCfg(Fin=F_IN, H=N_HEADS, CH=C_HID, NT=lay['NT'], NB=lay['NB'],
                    NT0=lay['NT0'], ROWPAD0=lay['ROWPAD0'], ROWPAD1=lay['ROWPAD1'],
                    relu=True, calc_residual=True, add_residual=False,
                    chunk=2048, bf16=True)
    nc1 = bacc.Bacc("TRN2", target_bir_lowering=False, debug=False,
                    num_devices=NCORES)
    build_layer(nc1, cfg1)
    nc1.compile()
    cfg2 = LayerCfg(Fin=C_HID, H=N_HEADS, CH=K_OUT, NT=lay['NT'], NB=lay['NB'],
                    NT0=lay['NT0'], ROWPAD0=lay['ROWPAD0'], ROWPAD1=lay['ROWPAD1'],
                    relu=False, calc_residual=False, add_residual=True,
                    chunk=2048, bf16=True, exb_pool=True, bufs_big=True)
    nc2 = bacc.Bacc("TRN2", target_bir_lowering=False, debug=False,
                    num_devices=NCORES)
    build_layer(nc2, cfg2)
    nc2.compile()
    _compiled_cache[key] = (nc1, nc2)
    return nc1, nc2


def _compact_tables(xfull, m, Fin, lay):
    outs = []
    for s, rp in ((0, lay['ROWPAD0']), (1, lay['ROWPAD1'])):
        xc = np.zeros((rp, Fin), np.float32)
        u = m['usrc'][s]
        xc[:len(u)] = xfull[u]
        outs.append(np.ascontiguousarray(xc.T.astype(_BF16NP)))
    return outs


def _local_table(xfull, Fin, c, lay):
    nloc, nloc_pad = lay['nloc'], lay['nloc_pad']
    d0 = c * nloc
    d1 = min((c + 1) * nloc, N_NODES)
    xl = np.zeros((nloc_pad, Fin), np.float32)
    xl[:d1 - d0] = xfull[d0:d1]
    return np.ascontiguousarray(xl.T)


def kernel(x, edge_index, xyz, lin1_w, lin1_b, wl1, wr1, att1, b1,
           wl2, wr2, att2, b2):
    from concourse.bass_utils import run_bass_kernel_spmd

    x = np.asarray(x, dtype=np.float32)
    edge_index = np.asarray(edge_index)
    metas, lay = preprocess_edges(edge_index, N_NODES, NCORES)
    nc1, nc2 = _build_programs(lay)
    nloc = lay['nloc']

    attb1, b1bc, iotar = host_constants(N_HEADS * C_HID, C_HID,
                                        np.asarray(att1), np.asarray(b1))
    attb2, b2bc, _ = host_constants(N_HEADS * K_OUT, K_OUT,
                                    np.asarray(att2), np.asarray(b2))
    linbbc = np.tile(np.asarray(lin1_b, dtype=np.float32).reshape(1, K_OUT),
                     (P, 1))

    in_maps1 = []
    for c in range(NCORES):
        xtc = _compact_tables(x, metas[c], F_IN, lay)
        in_maps1.append(dict(
            xTc0=xtc[0], xTc1=xtc[1], xTloc=_local_table(x, F_IN, c, lay),
            wl=np.asarray(wl1, dtype=np.float32).astype(_BF16NP),
            wr=np.asarray(wr1, dtype=np.float32),
            attb=attb1.astype(_BF16NP), bbc=b1bc, idx16=metas[c]['idx16'],
            erelc=metas[c]['erelc'], iotar=iotar,
            linw=np.asarray(lin1_w, dtype=np.float32), linb=linbbc))
    res1 = run_bass_kernel_spmd(nc1, in_maps1, core_ids=list(range(NCORES)))

    h_full = np.zeros((N_NODES, C_HID), np.float32)
    res_full = np.zeros((N_NODES, K_OUT), np.float32)
    for c in range(NCORES):
        d0 = c * nloc
        d1 = min((c + 1) * nloc, N_NODES)
        h_full[d0:d1] = res1.results[c]["out"][:d1 - d0]
        res_full[d0:d1] = res1.results[c]["resout"][:d1 - d0]

    in_maps2 = []
    for c in range(NCORES):
        d0 = c * nloc
        d1 = min((c + 1) * nloc, N_NODES)
        htc = _compact_tables(h_full, metas[c], C_HID, lay)
        resin = np.zeros((lay['nloc_pad'], K_OUT), np.float32)
        resin[:d1 - d0] = res_full[d0:d1]
        in_maps2.append(dict(
            xTc0=htc[0], xTc1=htc[1], xTloc=_local_table(h_full, C_HID, c, lay),
            wl=np.asarray(wl2, dtype=np.float32).astype(_BF16NP),
            wr=np.asarray(wr2, dtype=np.float32),
            attb=attb2.astype(_BF16NP), bbc=b2bc, idx16=metas[c]['idx16'],
            erelc=metas[c]['erelc'], iotar=iotar, resin=resin))
    res2 = run_bass_kernel_spmd(nc2, in_maps2, core_ids=list(range(NCORES)))

    out = np.zeros((N_NODES, K_OUT), np.float32)
    for c in range(NCORES):
        d0 = c * nloc
        d1 = min((c + 1) * nloc, N_NODES)
        out[d0:d1] = res2.results[c]["out"][:d1 - d0]
    return out



# revision 4
# speedup vs baseline: 1.5646x; 1.5646x over previous
"""Bass/Tile GATv2 kernel for TRN2, dst-sharded across 8 cores.

Two GATv2 layers (+linear residual), each run as one SPMD program over 8
NeuronCores. Host side: nodes are bin-packed into (core, tile, slot) so every
128-dst tile has nearly equal edge count (NB blocks of 128 edges); one-hot
edge->dst matrices (oh) and their transposes (ohT) are precomputed on host and
streamed to the device; source features are compacted per subphase so int16
gather indices suffice.

Device side per dst tile (all feature tensors channel-major: col = c*H + h):
  - DMA oh/ohT stream + dma_gather of xl rows (from a DRAM table built in
    phase 1 as x_compact @ wl)
  - xr = xtloc @ wr (PE) per tile
  - pass 1 per 128-edge block: s = ohT^T@xr + I@xlg (PE, PSUM), lr =
    prelu(s) (scalar), lr *= att (DVE 2x), fold+reduce -> e[:,8] (DVE)
  - one exp over all blocks' e (scalar)
  - pass 2 per block: xlw = xlg * exp-bcast (DVE 2x, ch-major), U += oh^T@xlw,
    D += oh^T@ex (PE)
  - epilogue: out = mean_h(U/D) + bias [+res / relu]
"""
from contextlib import ExitStack
from dataclasses import dataclass

import numpy as np

import concourse.bass as bass
import concourse.tile as tile
from concourse import bacc, mybir
from concourse.masks import make_identity

F32 = mybir.dt.float32
BF16 = mybir.dt.bfloat16
I16 = mybir.dt.int16
AF = mybir.ActivationFunctionType
ALU = mybir.AluOpType
AX = mybir.AxisListType
P = 128


@dataclass
class LayerCfg:
    Fin: int            # contraction dim of input features (128 L1, 64 L2)
    H: int              # heads
    CH: int             # per-head channels (64 L1, 32 L2)
    NT: int             # dst tiles per core
    NB: int             # edge blocks per tile (uniform across cores/tiles)
    NT0: int            # tiles in subphase 0 (rest in subphase 1)
    ROWPAD0: int        # compact-table rows, subphase 0 (128-mult, uniform)
    ROWPAD1: int        # compact-table rows, subphase 1
    relu: bool = False
    calc_residual: bool = False
    add_residual: bool = False
    RESC: int = 32
    neg_slope: float = 0.2
    chunk: int = 2048   # xTc column chunk for phase 1

    @property
    def HC(self):
        return self.H * self.CH


def build_layer(nc: bacc.Bacc, cfg: LayerCfg):
    HC, H, CH, Fin, NT, NB = cfg.HC, cfg.H, cfg.CH, cfg.Fin, cfg.NT, cfg.NB
    NLOC = NT * P
    NIDX = NB * P
    IDXW = NIDX // 16
    OHW = 2 * NB * P          # oh+ohT columns per tile

    dt = nc.dram_tensor
    xTc0_t = dt("xTc0", (Fin, cfg.ROWPAD0), BF16, kind="ExternalInput")
    xTc1_t = dt("xTc1", (Fin, cfg.ROWPAD1), BF16, kind="ExternalInput")
    xTloc_t = dt("xTloc", (Fin, NLOC), BF16, kind="ExternalInput")
    wl_t = dt("wl", (Fin, HC), BF16, kind="ExternalInput")
    wr_t = dt("wr", (Fin, HC), BF16, kind="ExternalInput")
    attb_t = dt("attb", (P, HC), BF16, kind="ExternalInput")
    bbc_t = dt("bbc", (P, CH), BF16, kind="ExternalInput")
    idx16_t = dt("idx16", (P, NT * IDXW), I16, kind="ExternalInput")
    ohall_t = dt("ohall", (P, NT * OHW), BF16, kind="ExternalInput")
    if cfg.calc_residual:
        linw_t = dt("linw", (Fin, cfg.RESC), BF16, kind="ExternalInput")
        linb_t = dt("linb", (P, cfg.RESC), BF16, kind="ExternalInput")
        resout_t = dt("resout", (NLOC, cfg.RESC), BF16, kind="ExternalOutput")
    if cfg.add_residual:
        resin_t = dt("resin", (NLOC, cfg.RESC), BF16, kind="ExternalInput")
    out_t = dt("out", (NLOC, CH), BF16, kind="ExternalOutput")
    xlc0_t = dt("xlc0", (cfg.ROWPAD0, HC), BF16)   # internal scratch
    xlc1_t = dt("xlc1", (cfg.ROWPAD1, HC), BF16)

    with tile.TileContext(nc) as tc, ExitStack() as ctx, \
            nc.allow_low_precision(reason="bf16 softmax scores within 2e-2 tol"):
        cpool = ctx.enter_context(tc.tile_pool(name="const", bufs=1))
        xt_pool = ctx.enter_context(tc.tile_pool(name="xt", bufs=3))
        cp_pool = ctx.enter_context(tc.tile_pool(name="cp", bufs=2))
        g_pool = ctx.enter_context(tc.tile_pool(name="g", bufs=2))
        ohs_pool = ctx.enter_context(tc.tile_pool(name="ohs", bufs=2))
        lr_pool = ctx.enter_context(tc.tile_pool(name="lr", bufs=3))
        sm_pool = ctx.enter_context(tc.tile_pool(name="sm", bufs=4))
        ps_pool = ctx.enter_context(tc.tile_pool(name="ps", bufs=3, space="PSUM"))
        psu_pool = ctx.enter_context(tc.tile_pool(name="psu", bufs=2, space="PSUM"))
        psd_pool = ctx.enter_context(tc.tile_pool(name="psd", bufs=2, space="PSUM"))

        # ---- constants ----
        wl_sb = cpool.tile([Fin, HC], BF16)
        nc.sync.dma_start(out=wl_sb[:], in_=wl_t[:, :])
        wr_sb = cpool.tile([Fin, HC], BF16)
        nc.sync.dma_start(out=wr_sb[:], in_=wr_t[:, :])
        attb_sb = cpool.tile([P, HC], BF16)
        nc.sync.dma_start(out=attb_sb[:], in_=attb_t[:, :])
        bbc_sb = cpool.tile([P, CH], BF16)
        nc.sync.dma_start(out=bbc_sb[:], in_=bbc_t[:, :])
        ident_sb = cpool.tile([P, P], BF16)
        make_identity(nc, ident_sb[:])
        idx16_sb = cpool.tile([P, NT * IDXW], I16)
        nc.sync.dma_start(out=idx16_sb[:], in_=idx16_t[:, :])
        xtloc_sb = cpool.tile([Fin, NLOC], BF16)
        nc.sync.dma_start(out=xtloc_sb[:], in_=xTloc_t[:, :])
        if cfg.calc_residual:
            linw_sb = cpool.tile([Fin, cfg.RESC], BF16)
            nc.sync.dma_start(out=linw_sb[:], in_=linw_t[:, :])
            linb_sb = cpool.tile([P, cfg.RESC], BF16)
            nc.sync.dma_start(out=linb_sb[:], in_=linb_t[:, :])
            res_acc = cpool.tile([P, NT * cfg.RESC], BF16)
        if cfg.add_residual:
            res_sb = cpool.tile([P, NT * cfg.RESC], BF16)
            nc.sync.dma_start(
                out=res_sb[:].rearrange("p (t c) -> p t c", t=NT),
                in_=resin_t[:, :].rearrange("(t p) c -> p t c", p=P),
            )
        h_acc = cpool.tile([P, NT * CH], BF16)

        # ---- phase 1: xlc = x_compact @ wl (per subphase) ----
        flip = 0
        for xTc_t, xlc_t, rows in ((xTc0_t, xlc0_t, cfg.ROWPAD0),
                                   (xTc1_t, xlc1_t, cfg.ROWPAD1)):
            c0 = 0
            while c0 < rows:
                csz = min(cfg.chunk, rows - c0)
                xt_sb = xt_pool.tile([Fin, csz], BF16, tag="xt")
                nc.sync.dma_start(out=xt_sb[:], in_=xTc_t[:, c0:c0 + csz])
                nj = csz // P
                ob = cp_pool.tile([P, nj * HC], BF16, tag="cp")
                for j in range(nj):
                    ps = ps_pool.tile([P, HC], F32, tag="mm")
                    nc.tensor.matmul(ps[:], lhsT=xt_sb[:, j * P:(j + 1) * P],
                                     rhs=wl_sb[:], start=True, stop=True)
                    dst = ob[:, j * HC:(j + 1) * HC]
                    if flip % 2 == 0:
                        nc.vector.tensor_copy(dst, ps[:])
                    else:
                        nc.scalar.copy(dst, ps[:])
                    flip += 1
                nc.sync.dma_start(
                    out=xlc_t[c0:c0 + csz, :].rearrange("(j p) c -> p j c", p=P),
                    in_=ob[:].rearrange("p (j c) -> p j c", j=nj))
                c0 += csz

        # ---- phase 2: residual (layer 1 only) ----
        if cfg.calc_residual:
            for t in range(NT):
                ps2 = psd_pool.tile([P, cfg.RESC], F32, tag="D")
                nc.tensor.matmul(ps2[:], lhsT=xtloc_sb[:, t * P:(t + 1) * P],
                                 rhs=linw_sb[:], start=True, stop=True)
                nc.vector.tensor_tensor(
                    res_acc[:, t * cfg.RESC:(t + 1) * cfg.RESC],
                    ps2[:], linb_sb[:], op=ALU.add)

        # ---- phase 3: edge processing ----
        for t in range(NT):
            src_tab = xlc0_t if t < cfg.NT0 else xlc1_t
            ohs = ohs_pool.tile([P, OHW], BF16, tag="ohs")
            nc.sync.dma_start(out=ohs[:], in_=ohall_t[:, t * OHW:(t + 1) * OHW])
            xlg = g_pool.tile([P, NB * HC], BF16, tag="g")
            nc.gpsimd.dma_gather(
                out_ap=xlg[:].rearrange("p (b c) -> p b c", b=NB),
                in_ap=src_tab[:, :],
                idxs_ap=idx16_sb[:, t * IDXW:(t + 1) * IDXW],
                num_idxs=NIDX,
                num_idxs_reg=NIDX,
                elem_size=HC,
                single_packet=False,
            )
            psx = ps_pool.tile([P, HC], F32, tag="mm")
            nc.tensor.matmul(psx[:], lhsT=xtloc_sb[:, t * P:(t + 1) * P],
                             rhs=wr_sb[:], start=True, stop=True)
            xrt = lr_pool.tile([P, HC], BF16, tag="xrt")
            nc.scalar.copy(xrt[:], psx[:])

            eacc = sm_pool.tile([P, NB * H], BF16, tag="eacc")
            # oh layout per tile: [ohT(NB blocks) | oh(NB blocks)]
            for b in range(NB):
                s = ps_pool.tile([P, HC], F32, tag="mm")
                nc.tensor.matmul(s[:], lhsT=ohs[:, b * P:(b + 1) * P],
                                 rhs=xrt[:], start=True, stop=False)
                nc.tensor.matmul(s[:], lhsT=ident_sb[:],
                                 rhs=xlg[:, b * HC:(b + 1) * HC],
                                 start=False, stop=True)
                lr = lr_pool.tile([P, HC], BF16, tag="lr")
                nc.scalar.activation(lr[:], s[:], AF.Prelu, alpha=cfg.neg_slope)
                nc.vector.tensor_tensor(lr[:], lr[:], attb_sb[:], op=ALU.mult)
                half = sm_pool.tile([P, HC // 2], BF16, tag="half")
                nc.vector.tensor_tensor(half[:], lr[:, :HC // 2],
                                        lr[:, HC // 2:], op=ALU.add)
                nc.vector.tensor_reduce(
                    eacc[:, b * H:(b + 1) * H],
                    half[:].rearrange("p (c h) -> p h c", h=H),
                    axis=AX.X, op=ALU.add)
            exa = sm_pool.tile([P, NB * H], BF16, tag="exa")
            nc.scalar.activation(exa[:], eacc[:], AF.Exp)
            U = psu_pool.tile([P, HC], F32, tag="U")
            D = psd_pool.tile([P, H], F32, tag="D")
            for b in range(NB):
                xb = xlg[:, b * HC:(b + 1) * HC].rearrange("p (c h) -> p c h", h=H)
                exm = exa[:, b * H:(b + 1) * H].rearrange(
                    "p (o h) -> p o h", o=1).broadcast_to([P, CH, H])
                nc.vector.tensor_tensor(xb, xb, exm, op=ALU.mult)
                nc.tensor.matmul(U[:], lhsT=ohs[:, (NB + b) * P:(NB + b + 1) * P],
                                 rhs=xlg[:, b * HC:(b + 1) * HC],
                                 start=(b == 0), stop=(b == NB - 1))
                nc.tensor.matmul(D[:], lhsT=ohs[:, (NB + b) * P:(NB + b + 1) * P],
                                 rhs=exa[:, b * H:(b + 1) * H],
                                 start=(b == 0), stop=(b == NB - 1))
            # epilogue for tile t
            dsafe = sm_pool.tile([P, H], F32, tag="dsafe")
            nc.vector.tensor_scalar_max(dsafe[:], D[:], 1e-30)
            rcp = sm_pool.tile([P, H], F32, tag="rcp")
            nc.vector.reciprocal(rcp[:], dsafe[:])
            rcpb = sm_pool.tile([P, H], BF16, tag="rcpb")
            nc.vector.tensor_scalar_mul(rcpb[:], rcp[:], 1.0 / H)
            au = lr_pool.tile([P, HC], BF16, tag="au")
            nc.scalar.copy(au[:], U[:])
            nc.vector.tensor_tensor(
                au[:].rearrange("p (c h) -> p c h", h=H),
                au[:].rearrange("p (c h) -> p c h", h=H),
                rcpb[:].rearrange("p (o h) -> p o h", o=1).broadcast_to([P, CH, H]),
                op=ALU.mult)
            t1 = sm_pool.tile([P, CH], BF16, tag="t1")
            nc.vector.tensor_reduce(
                t1[:], au[:].rearrange("p (c h) -> p c h", h=H),
                axis=AX.X, op=ALU.add)
            hslice = h_acc[:, t * CH:(t + 1) * CH]
            if cfg.add_residual:
                nc.vector.tensor_tensor(t1[:], t1[:],
                                        res_sb[:, t * cfg.RESC:(t + 1) * cfg.RESC],
                                        op=ALU.add)
            nc.vector.tensor_tensor(t1[:], t1[:], bbc_sb[:], op=ALU.add)
            if cfg.relu:
                nc.scalar.activation(hslice, t1[:], AF.Relu)
            else:
                nc.vector.tensor_copy(hslice, t1[:])

        # ---- final stores ----
        nc.sync.dma_start(
            out=out_t[:, :].rearrange("(t p) c -> p t c", p=P),
            in_=h_acc[:].rearrange("p (t c) -> p t c", t=NT),
        )
        if cfg.calc_residual:
            nc.sync.dma_start(
                out=resout_t[:, :].rearrange("(t p) c -> p t c", p=P),
                in_=res_acc[:].rearrange("p (t c) -> p t c", t=NT),
            )
    return nc


# ---------------------------------------------------------------------------
# Host-side preprocessing
# ---------------------------------------------------------------------------

def pack_nodes(edge_index: np.ndarray, n: int, ncores: int, nt: int):
    """Assign each node to (core, tile, slot) balancing per-tile edge load.

    Returns (node_core, node_tile, node_slot, per-tile edge caps honored NB).
    """
    import heapq
    deg = np.bincount(edge_index[1], minlength=n).astype(np.int64) + 1
    nbins = ncores * nt
    for NB in (9, 10, 11):
        cap = NB * P
        order = np.argsort(-deg, kind="stable")
        loads = np.zeros(nbins, np.int64)
        slots = np.zeros(nbins, np.int32)
        node_bin = np.full(n, -1, np.int32)
        heap = [(0, b) for b in range(nbins)]
        heapq.heapify(heap)
        ok = True
        skipped = []
        for node in order:
            d = int(deg[node])
            tried = []
            placed = False
            while heap:
                load, b = heapq.heappop(heap)
                if load != loads[b] or slots[b] >= P:
                    continue   # stale or full
                if load + d <= cap:
                    node_bin[node] = b
                    loads[b] += d
                    slots[b] += 1
                    if slots[b] < P:
                        heapq.heappush(heap, (loads[b], b))
                    placed = True
                    break
                tried.append((load, b))
            for item in tried:
                heapq.heappush(heap, item)
            if not placed:
                ok = False
                break
        if ok:
            bins = node_bin
            node_core = bins // nt
            node_tile = bins % nt
            node_slot = np.zeros(n, np.int32)
            for b in range(nbins):
                idx = np.where(bins == b)[0]
                node_slot[idx] = np.arange(len(idx), dtype=np.int32)
            return node_core.astype(np.int32), node_tile.astype(np.int32), \
                node_slot, NB
    raise RuntimeError("packing failed")


def preprocess_edges(edge_index: np.ndarray, n: int, ncores: int,
                     nsub: int = 2, int16_cap: int = 32000):
    """Shard edges by packed dst; build per-core idx16 / oh streams.

    Returns (metas, layout). metas[c]:
      idx16  [128, NT*NB*8]  int16
      ohall  [128, NT*2*NB*128]  float32 (cast to bf16 later)
      usrc   [nsub] unique source-node arrays
      nodes  [NT*128] int64 node id per (tile,slot), -1 for empty
    """
    nloc = -(-n // ncores)
    NT = -(-nloc // P)
    node_core, node_tile, node_slot, NB = pack_nodes(edge_index, n, ncores, NT)
    NT0 = (NT + 1) // 2
    NIDX = NB * P
    IDXW = NIDX // 16

    loops = np.arange(n, dtype=np.int64)
    src = np.concatenate([edge_index[0].astype(np.int64), loops])
    dst = np.concatenate([edge_index[1].astype(np.int64), loops])
    ecore = node_core[dst]
    etile = node_tile[dst]
    eslot = node_slot[dst]

    metas = []
    rowmax = [0] * nsub
    for c in range(ncores):
        sel = ecore == c
        s_c, t_c, d_c = src[sel], etile[sel], eslot[sel]
        order = np.argsort(t_c, kind="stable")
        s_c, t_c, d_c = s_c[order], t_c[order], d_c[order]
        tcnt = np.bincount(t_c, minlength=NT)
        tstart = np.concatenate([[0], np.cumsum(tcnt)])

        idxflat = np.zeros((NT, NIDX), np.int64)
        oh = np.zeros((NT, NB, P, P), np.float32)      # [t, b, e, d]
        usrcs = []
        for sub in range(nsub):
            tlo, thi = (0, NT0) if sub == 0 else (NT0, NT)
            allsrc = s_c[tstart[tlo]:tstart[thi]]
            usrc, inv = np.unique(allsrc, return_inverse=True)
            assert len(usrc) < int16_cap, f"int16 cap exceeded: {len(usrc)}"
            usrcs.append(usrc)
            comp = np.zeros(len(s_c), np.int64)
            comp[tstart[tlo]:tstart[thi]] = inv
            for t in range(tlo, thi):
                ne = tcnt[t]
                e0 = tstart[t]
                idxflat[t, :ne] = comp[e0:e0 + ne]
                eix = np.arange(ne)
                oh[t, eix // P, eix % P, d_c[e0:e0 + ne]] = 1.0
        ohT = np.transpose(oh, (0, 1, 3, 2))           # [t, b, d, e]
        # per tile: [ohT blocks | oh blocks] -> [t, 2, NB, P(row), P(col)]
        ohcat = np.stack([ohT, oh], axis=1)            # [t, 2, b, row, col]
        ohall = np.ascontiguousarray(
            np.transpose(ohcat, (3, 0, 1, 2, 4)).reshape(P, NT * 2 * NB * P))

        w = idxflat.reshape(NT, IDXW, 16).transpose(0, 2, 1)
        idx16 = np.tile(w, (1, 8, 1)).transpose(1, 0, 2).reshape(P, NT * IDXW)

        nodes = np.full(NT * P, -1, np.int64)
        nsel = np.where(node_core == c)[0]
        nodes[node_tile[nsel] * P + node_slot[nsel]] = nsel
        metas.append(dict(idx16=idx16.astype(np.int16), ohall=ohall,
                          usrc=usrcs, nodes=nodes))
        for sub in range(nsub):
            rowmax[sub] = max(rowmax[sub], len(metas[c]['usrc'][sub]))
    rowpad = [max(P, -(-r // P) * P) for r in rowmax]
    layout = dict(NT=NT, NB=NB, NT0=NT0, nloc_pad=NT * P,
                  ROWPAD0=rowpad[0], ROWPAD1=rowpad[1])
    return metas, layout


# ---------------------------------------------------------------------------
# Top-level kernel entry: full inputs -> full output, 8 NeuronCores
# ---------------------------------------------------------------------------
import ml_dtypes

_BF16NP = ml_dtypes.bfloat16
N_NODES = 50000
F_IN = 128
N_HEADS = 8
C_HID = 64
K_OUT = 32
NCORES = 8

_compiled_cache = {}


def layer_cfgs(lay):
    common = dict(NT=lay['NT'], NB=lay['NB'], NT0=lay['NT0'],
                  ROWPAD0=lay['ROWPAD0'], ROWPAD1=lay['ROWPAD1'])
    cfg1 = LayerCfg(Fin=F_IN, H=N_HEADS, CH=C_HID, relu=True,
                    calc_residual=True, add_residual=False, **common)
    cfg2 = LayerCfg(Fin=C_HID, H=N_HEADS, CH=K_OUT, relu=False,
                    calc_residual=False, add_residual=True, **common)
    return cfg1, cfg2


def _build_programs(lay):
    key = (lay['NT'], lay['NB'], lay['NT0'], lay['ROWPAD0'], lay['ROWPAD1'])
    if key in _compiled_cache:
        return _compiled_cache[key]
    cfg1, cfg2 = layer_cfgs(lay)
    ncs = []
    for cfg in (cfg1, cfg2):
        nc = bacc.Bacc("TRN2", target_bir_lowering=False, debug=False,
                       num_devices=NCORES)
        build_layer(nc, cfg)
        nc.compile()
        ncs.append(nc)
    _compiled_cache[key] = tuple(ncs)
    return _compiled_cache[key]


def _ch_major_cols(H, C):
    """col j holds original column h*C+c with j = c*H + h."""
    j = np.arange(H * C)
    c, h = j // H, j % H
    return h * C + c


def _compact_tables(xfull, m, Fin, lay):
    outs = []
    for s, rp in ((0, lay['ROWPAD0']), (1, lay['ROWPAD1'])):
        xc = np.zeros((rp, Fin), np.float32)
        u = m['usrc'][s]
        xc[:len(u)] = xfull[u]
        outs.append(np.ascontiguousarray(xc.T).astype(_BF16NP))
    return outs


def _local_table(xfull, Fin, m):
    nodes = m['nodes']
    xl = np.zeros((len(nodes), Fin), np.float32)
    valid = nodes >= 0
    xl[valid] = xfull[nodes[valid]]
    return np.ascontiguousarray(xl.T).astype(_BF16NP)


def kernel(x, edge_index, xyz, lin1_w, lin1_b, wl1, wr1, att1, b1,
           wl2, wr2, att2, b2):
    from concourse.bass_utils import run_bass_kernel_spmd

    x = np.asarray(x, dtype=np.float32)
    edge_index = np.asarray(edge_index)
    metas, lay = preprocess_edges(edge_index, N_NODES, NCORES)
    nc1, nc2 = _build_programs(lay)

    perm1 = _ch_major_cols(N_HEADS, C_HID)
    perm2 = _ch_major_cols(N_HEADS, K_OUT)
    wl1_cm = np.asarray(wl1, np.float32)[:, perm1]
    wr1_cm = np.asarray(wr1, np.float32)[:, perm1]
    att1_cm = np.asarray(att1, np.float32).reshape(-1)[perm1]
    wl2_cm = np.asarray(wl2, np.float32)[:, perm2]
    wr2_cm = np.asarray(wr2, np.float32)[:, perm2]
    att2_cm = np.asarray(att2, np.float32).reshape(-1)[perm2]

    attb1 = np.tile(att1_cm.reshape(1, -1), (P, 1)).astype(_BF16NP)
    attb2 = np.tile(att2_cm.reshape(1, -1), (P, 1)).astype(_BF16NP)
    b1bc = np.tile(np.asarray(b1, np.float32).reshape(1, -1), (P, 1)).astype(_BF16NP)
    b2bc = np.tile(np.asarray(b2, np.float32).reshape(1, -1), (P, 1)).astype(_BF16NP)
    linbbc = np.tile(np.asarray(lin1_b, np.float32).reshape(1, -1),
                     (P, 1)).astype(_BF16NP)

    in_maps1 = []
    for c in range(NCORES):
        m = metas[c]
        xtc = _compact_tables(x, m, F_IN, lay)
        in_maps1.append(dict(
            xTc0=xtc[0], xTc1=xtc[1], xTloc=_local_table(x, F_IN, m),
            wl=wl1_cm.astype(_BF16NP), wr=wr1_cm.astype(_BF16NP),
            attb=attb1, bbc=b1bc, idx16=m['idx16'],
            ohall=m['ohall'].astype(_BF16NP),
            linw=np.asarray(lin1_w, np.float32).astype(_BF16NP), linb=linbbc))
    res1 = run_bass_kernel_spmd(nc1, in_maps1, core_ids=list(range(NCORES)))

    h_full = np.zeros((N_NODES, C_HID), np.float32)
    res_full = np.zeros((N_NODES, K_OUT), np.float32)
    for c in range(NCORES):
        nodes = metas[c]['nodes']
        valid = nodes >= 0
        h_full[nodes[valid]] = np.asarray(
            res1.results[c]["out"], np.float32)[valid]
        res_full[nodes[valid]] = np.asarray(
            res1.results[c]["resout"], np.float32)[valid]

    in_maps2 = []
    for c in range(NCORES):
        m = metas[c]
        htc = _compact_tables(h_full, m, C_HID, lay)
        resin = np.zeros((lay['nloc_pad'], K_OUT), np.float32)
        nodes = m['nodes']
        valid = nodes >= 0
        resin[valid] = res_full[nodes[valid]]
        in_maps2.append(dict(
            xTc0=htc[0], xTc1=htc[1], xTloc=_local_table(h_full, C_HID, m),
            wl=wl2_cm.astype(_BF16NP), wr=wr2_cm.astype(_BF16NP),
            attb=attb2, bbc=b2bc, idx16=m['idx16'],
            ohall=m['ohall'].astype(_BF16NP),
            resin=resin.astype(_BF16NP)))
    res2 = run_bass_kernel_spmd(nc2, in_maps2, core_ids=list(range(NCORES)))

    out = np.zeros((N_NODES, K_OUT), np.float32)
    for c in range(NCORES):
        nodes = metas[c]['nodes']
        valid = nodes >= 0
        out[nodes[valid]] = np.asarray(
            res2.results[c]["out"], np.float32)[valid]
    return out
